# revision 3
# baseline (speedup 1.0000x reference)
"""Trainium2 Bass kernel for BlockChunkedActivityRoutedNet.

Reference computation (B=4096, IN_F=4096, 8 chunks of 512, top-2 by mean|x|,
chunk-expert Linears 512->512, concat -> final Linear 1024->4096):

    xr = x.reshape(B, 8, 512)
    activities = mean(|xr|, axis=(0, 2))            # over the WHOLE batch
    i0, i1 = top2(activities)                        # descending
    h = concat(xr[:, i0] @ Wc[i0] + bc[i0], xr[:, i1] @ Wc[i1] + bc[i1])
    out = h @ W_final + b_final

Distribution: data-parallel over the batch across 8 NeuronCores (512 rows
each). Per-chunk |x| partial sums are AllReduced (tiny [1,8] collective) so
every core computes the identical top-2 routing.

Schedule (v3):
  - Head: x load as 8 x 512KB chunk DMAs on the sync HWDGE ring; per-chunk
    |.| reduces split DVE (even chunks) / ScalarE (odd) as tiles land;
    partition-reduce via ones-matmul (the FIRST op in the Tensor stream so
    nothing shallow-dep can block the AllReduce trigger chain); AllReduce
    trigger lands ~23us.
  - b_final broadcast [128,4096] and b_chunks transpose [128,32] are
    HOST-side layout prep, uploaded directly - no PE/DVE work competes with
    the trigger chain (in v2 Tile's list scheduler hoisted those matmuls
    ahead of the partition-reduce and the trigger slipped 25us).
  - W_final prefetch (8 x 1MB row-blocks) + bfin_bc load fill the
    AllReduce/barrier wait window.
  - Routing: top2 via vector.max/max_index, K=1 ones-matmul broadcast,
    iota row offsets. Selected rows gathered from a combined [4096, 1024]
    bf16 table XW (x row | W row share the row index): 8 indirect DMAs of
    [128, 1024], chunk 0 first.
  - L1 hT[s][d] via 4 k-tile matmuls into PSUM, bias added on ScalarE
    eviction to bf16. Six L2 PSUM groups (bt=0, o=0..5, kf 0-3 on hT[0])
    pre-start while chunk-1 gathers/L1 run so the PE never idles.
  - L2 loops bt outer / o inner; per bt the 8 o-groups evict (DVE add of
    b_final, cast to bf16) into one [128, 4096] staging tile, stored with a
    single 1MB DMA -> 4 output DMAs total, small tail. Host casts to fp32.
"""

import numpy as np
import ml_dtypes

import concourse.bass as bass
import concourse.bacc as bacc
import concourse.mybir as mybir
from concourse.tile import TileContext
from concourse.bass_utils import run_bass_kernel_spmd

dt = mybir.dt
P = 128

NUM_CHUNKS = 8
TOP_K = 2
IN_F = 4096
HID_F = 4096
OUT_F = 4096
B = 4096
CIN = IN_F // NUM_CHUNKS      # 512
COUT = HID_F // NUM_CHUNKS    # 512
N_CORES = 8
BS = B // N_CORES             # 512 rows per core

BT = BS // P                  # 4 batch tiles per core
KT = CIN // P                 # 4 k-tiles per selected chunk
DT_ = COUT // P               # 4 d-tiles per selected chunk
KF = TOP_K * DT_              # 8 k-tiles for the final matmul
OT = OUT_F // 512             # 8 output column tiles of 512

_cache = {}


def _build():
    nc = bacc.Bacc(num_devices=N_CORES, name="chunk_routed_net",
                   num_swdge_queues=4)

    xT = nc.dram_tensor("xT_shard", [IN_F, BS], dt.bfloat16,
                        kind="ExternalInput")
    XW = nc.dram_tensor("XW_shard", [IN_F, BS + COUT], dt.bfloat16,
                        kind="ExternalInput")
    bT_d = nc.dram_tensor("bT_host", [P, DT_ * NUM_CHUNKS], dt.float32,
                          kind="ExternalInput")
    Wf = nc.dram_tensor("W_final", [COUT * TOP_K, OUT_F], dt.bfloat16,
                        kind="ExternalInput")
    bfb = nc.dram_tensor("b_final_bc", [P, OUT_F], dt.float32,
                         kind="ExternalInput")
    out = nc.dram_tensor("out_shard", [BS, OUT_F], dt.bfloat16,
                         kind="ExternalOutput")

    with TileContext(nc) as tc:
        with tc.tile_pool(name="consts", bufs=1) as consts, \
             tc.tile_pool(name="route", bufs=1) as route, \
             tc.tile_pool(name="xl", bufs=1) as xl_pool, \
             tc.tile_pool(name="gath", bufs=1) as gath, \
             tc.tile_pool(name="hts", bufs=1) as hts, \
             tc.tile_pool(name="bfinp", bufs=1) as bfinp, \
             tc.tile_pool(name="wfs", bufs=8) as wfs, \
             tc.tile_pool(name="outs", bufs=2) as outs, \
             tc.tile_pool(name="dram", bufs=1, space="DRAM") as dram:

            # ---------------- constants ----------------
            ones_col = consts.tile([P, 1], dt.float32)     # partition reduce
            nc.vector.memset(ones_col[:], 1.0)
            ones_k1 = consts.tile([1, P], dt.float32)      # K=1 bcast matmul
            nc.vector.memset(ones_k1[:], 1.0)
            # C_W[p, j] = p + 128*j       (row offset within a chunk)
            C_W = consts.tile([P, KT], dt.int32)
            nc.gpsimd.iota(C_W[:], pattern=[[P, KT]], base=0, channel_multiplier=1)
            C_Wf = consts.tile([P, KT], dt.float32)
            nc.vector.tensor_copy(C_Wf[:], C_W[:])
            # C8[p, c] = c                (chunk-id iota along free dim)
            C8 = consts.tile([P, NUM_CHUNKS], dt.int32)
            nc.gpsimd.iota(C8[:], pattern=[[1, NUM_CHUNKS]], base=0,
                           channel_multiplier=0)
            C8f = consts.tile([P, NUM_CHUNKS], dt.float32)
            nc.vector.tensor_copy(C8f[:], C8[:])

            with tc.tile_pool(name="ps_early", bufs=2, space="PSUM") as ps_early:
                # ------------ activities from xT (chunk-aligned loads) -----
                # chunk c rows of view [1024, 2048] = xT feats 4r..4r+3; rows
                # c*128..c*128+127 are exactly chunk c -> 4KB runs/partition.
                xT_w = xT[:].rearrange("(r q) b -> r (q b)", q=4)  # [1024,2048]
                actcol = route.tile([P, NUM_CHUNKS], dt.float32)
                scr = route.tile([P, 4 * BS], dt.bfloat16)  # ACT throwaway
                xls = []
                for c in range(NUM_CHUNKS):
                    xlt = xl_pool.tile([P, 4 * BS], dt.bfloat16, tag=f"xl{c}",
                                       name=f"xl{c}")
                    nc.sync.dma_start(xlt[:], xT_w[c * P:(c + 1) * P, :])
                    xls.append(xlt)
                for c in range(NUM_CHUNKS):
                    if c % 2 == 0:
                        nc.vector.tensor_reduce(
                            actcol[:, c:c + 1], xls[c][:],
                            axis=mybir.AxisListType.X, op=mybir.AluOpType.add,
                            apply_absolute_value=True)
                    else:
                        nc.scalar.activation(
                            scr[:], xls[c][:],
                            mybir.ActivationFunctionType.Abs,
                            accum_out=actcol[:, c:c + 1])
                act_ps = ps_early.tile([1, NUM_CHUNKS], dt.float32, tag="psa")
                nc.tensor.matmul(act_ps[:], ones_col[:], actcol[:],
                                 start=True, stop=True)
                act_l = route.tile([1, NUM_CHUNKS], dt.float32)
                nc.scalar.copy(act_l[:], act_ps[:])

                # ------------ AllReduce ------------
                cc_in = dram.tile([1, NUM_CHUNKS], dt.float32)
                cc_out = dram.tile([1, NUM_CHUNKS], dt.float32)
                nc.sync.dma_start(cc_in[:], act_l[:])
                nc.gpsimd.collective_compute(
                    "AllReduce", mybir.AluOpType.add,
                    replica_groups=[list(range(N_CORES))],
                    ins=[cc_in.opt()], outs=[cc_out.opt()])

                # ---- work that fills the AllReduce wait ----
                # W_final prefetch: 8 x 1MB row-blocks [128, 4096]
                wf_tiles = []
                for kf in range(KF):
                    w = wfs.tile([P, OUT_F], dt.bfloat16, tag="wf",
                                 name=f"wf{kf}")
                    nc.sync.dma_start(w[:], Wf[kf * P:(kf + 1) * P, :])
                    wf_tiles.append(w)
                # host-prepped b_final broadcast + b_chunks transpose
                bfin_bc = bfinp.tile([P, OUT_F], dt.float32)
                nc.scalar.dma_start(bfin_bc[:], bfb[:])
                bT = route.tile([P, DT_ * NUM_CHUNKS], dt.float32)
                nc.scalar.dma_start(bT[:], bT_d[:])

                act_g = route.tile([1, NUM_CHUNKS], dt.float32)
                nc.scalar.dma_start(act_g[:], cc_out[:])

                # ------------ top-2 ------------
                maxv = route.tile([1, NUM_CHUNKS], dt.float32)
                maxi = route.tile([1, NUM_CHUNKS], dt.uint32)
                nc.vector.max(maxv[:], act_g[:])
                nc.vector.max_index(maxi[:], maxv[:], act_g[:])
                maxi_f = route.tile([1, NUM_CHUNKS], dt.float32)
                nc.vector.tensor_copy(maxi_f[:], maxi[:])

                # bcast[p, j] = idx[j] on every partition (K=1 matmul)
                bc_ps = ps_early.tile([P, NUM_CHUNKS], dt.float32, tag="psc")
                nc.tensor.matmul(bc_ps[:], ones_k1[:], maxi_f[:],
                                 start=True, stop=True)
                bcast = route.tile([P, NUM_CHUNKS], dt.float32)
                nc.vector.tensor_copy(bcast[:], bc_ps[:])

            # gather offsets: offW[p, s*4+kt] = sel_s*512 + kt*128 + p
            bc512 = route.tile([P, TOP_K], dt.float32)
            nc.vector.tensor_scalar_mul(bc512[:], bcast[:, 0:TOP_K], 512.0)
            offW_f = route.tile([P, TOP_K * KT], dt.float32)
            for s in range(TOP_K):
                nc.vector.tensor_scalar(
                    offW_f[:, s * KT:(s + 1) * KT], C_Wf[:],
                    bc512[:, s:s + 1], scalar2=None, op0=mybir.AluOpType.add)
            offW = route.tile([P, TOP_K * KT], dt.int32)
            nc.vector.tensor_copy(offW[:], offW_f[:])

            # ------------ gathers from combined XW table, chunk 0 first ---
            XWg = [[gath.tile([P, BS + COUT], dt.bfloat16, tag=f"xw{s}_{kt}",
                              name=f"xw{s}_{kt}")
                    for kt in range(KT)] for s in range(TOP_K)]
            for s in range(TOP_K):
                for kt in range(KT):
                    nc.gpsimd.indirect_dma_start(
                        out=XWg[s][kt][:], out_offset=None,
                        in_=XW[:],
                        in_offset=bass.IndirectOffsetOnAxis(
                            ap=offW[:, s * KT + kt:s * KT + kt + 1], axis=0))

            def xg(s, kt):
                return XWg[s][kt][:, 0:BS]          # [128, 512] x rows

            def wk(s, kt, d):
                return XWg[s][kt][:, BS + d * P:BS + (d + 1) * P]

            # chunk-bias select: bias[s][d][p] = bT[p, d*8 + sel_s]
            onehot = route.tile([P, TOP_K * NUM_CHUNKS], dt.float32)
            for s in range(TOP_K):
                nc.vector.tensor_scalar(
                    onehot[:, s * NUM_CHUNKS:(s + 1) * NUM_CHUNKS], C8f[:],
                    bcast[:, s:s + 1], scalar2=None,
                    op0=mybir.AluOpType.is_equal)
            bsel = [[route.tile([P, 1], dt.float32, tag=f"bs{s}_{d}",
                                name=f"bs{s}_{d}")
                     for d in range(DT_)] for s in range(TOP_K)]
            btmp = route.tile([P, NUM_CHUNKS], dt.float32)
            for s in range(TOP_K):
                for d in range(DT_):
                    nc.vector.tensor_tensor(
                        out=btmp[:], in0=bT[:, d * NUM_CHUNKS:(d + 1) * NUM_CHUNKS],
                        in1=onehot[:, s * NUM_CHUNKS:(s + 1) * NUM_CHUNKS],
                        op=mybir.AluOpType.mult)
                    nc.vector.tensor_reduce(
                        bsel[s][d][:], btmp[:], axis=mybir.AxisListType.X,
                        op=mybir.AluOpType.add)

            with tc.tile_pool(name="ps_h", bufs=2, space="PSUM") as ps_h, \
                 tc.tile_pool(name="ps_o", bufs=6, space="PSUM") as ps_o:
                # ------------ L1: hT[s][d] = (x_sel @ Wc[sel]).T + b -------
                hT = [[hts.tile([P, BS], dt.bfloat16, tag=f"ht{s}_{d}",
                                name=f"ht{s}_{d}")
                       for d in range(DT_)] for s in range(TOP_K)]

                def l1_chunk(s):
                    for d in range(DT_):
                        ph = ps_h.tile([P, BS], dt.float32, tag="ph",
                                       name=f"ph{s}_{d}")
                        for kt in range(KT):
                            nc.tensor.matmul(
                                ph[:], wk(s, kt, d), xg(s, kt),
                                start=(kt == 0), stop=(kt == KT - 1))
                        nc.scalar.activation(
                            hT[s][d][:], ph[:],
                            mybir.ActivationFunctionType.Identity,
                            bias=bsel[s][d][:, 0:1])

                l1_chunk(0)

                # pre-start six psum groups (bt=0, o=0..5) on chunk-0 hT
                # while chunk-1 gathers are still in flight (PE executes in
                # program order, so this fills the wait)
                PRE = [(0, 0), (0, 1), (0, 2), (0, 3), (0, 4), (0, 5)]
                pre = {}
                for (bt, po_) in PRE:
                    po = ps_o.tile([P, 512], dt.float32, tag="po",
                                   name=f"po_pre{bt}_{po_}")
                    for kf in range(DT_):
                        nc.tensor.matmul(
                            po[:], hT[0][kf][:, bt * P:(bt + 1) * P],
                            wf_tiles[kf][:, po_ * 512:(po_ + 1) * 512],
                            start=(kf == 0), stop=False)
                    pre[(bt, po_)] = po

                l1_chunk(1)

                # ------------ L2: out = h @ W_final + b_final --------------
                # bt outer / o inner; evict each o-group into a [128, 4096]
                # staging row, store once per bt (4 x 1MB output DMAs).
                for bt in range(BT):
                    ot_sb = outs.tile([P, OUT_F], dt.bfloat16, tag="ot",
                                      name=f"ot{bt}")
                    for o in range(OT):
                        osl = slice(o * 512, (o + 1) * 512)
                        if (bt, o) in pre:
                            po = pre[(bt, o)]
                            kfs = range(DT_, KF)
                        else:
                            po = ps_o.tile([P, 512], dt.float32, tag="po",
                                           name=f"po{bt}_{o}")
                            kfs = range(KF)
                        for kf in kfs:
                            s, d = divmod(kf, DT_)
                            nc.tensor.matmul(
                                po[:], hT[s][d][:, bt * P:(bt + 1) * P],
                                wf_tiles[kf][:, osl],
                                start=(kf == 0), stop=(kf == KF - 1))
                        nc.vector.tensor_tensor(
                            out=ot_sb[:, osl], in0=po[:], in1=bfin_bc[:, osl],
                            op=mybir.AluOpType.add)
                    nc.sync.dma_start(out[bt * P:(bt + 1) * P, :], ot_sb[:])
    nc.compile()
    return nc


def kernel(x, W_chunks, b_chunks, W_final, b_final):
    bf16 = ml_dtypes.bfloat16
    x = np.asarray(x, dtype=np.float32).astype(bf16)
    W_chunks = np.asarray(W_chunks, dtype=np.float32).astype(bf16)
    W_final = np.ascontiguousarray(
        np.asarray(W_final, dtype=np.float32).astype(bf16))
    b_chunks = np.asarray(b_chunks, dtype=np.float32)
    b_final = np.asarray(b_final, dtype=np.float32).reshape(OUT_F)

    # host-side layout prep (input-independent):
    # bT[p, d*8 + c] = b_chunks[c, d*128 + p]
    bT_host = np.ascontiguousarray(
        b_chunks.T.reshape(DT_, P, NUM_CHUNKS).transpose(1, 0, 2)
        .reshape(P, DT_ * NUM_CHUNKS))
    b_final_bc = np.ascontiguousarray(
        np.broadcast_to(b_final[None, :], (P, OUT_F)))

    if "nc" not in _cache:
        _cache["nc"] = _build()
    nc = _cache["nc"]

    Wc_rows = W_chunks.reshape(IN_F, COUT)          # [4096, 512] bf16
    in_maps = []
    for c in range(N_CORES):
        xT = np.ascontiguousarray(x[c * BS:(c + 1) * BS].T)  # [4096, 512]
        XW = np.concatenate([xT, Wc_rows], axis=1)           # [4096, 1024]
        in_maps.append({
            "xT_shard": xT,
            "XW_shard": np.ascontiguousarray(XW),
            "bT_host": bT_host,
            "W_final": W_final,
            "b_final_bc": b_final_bc,
        })

    res = run_bass_kernel_spmd(nc, in_maps, core_ids=list(range(N_CORES)))
    kernel.last_result = res
    return np.concatenate(
        [res.results[c]["out_shard"].astype(np.float32)
         for c in range(N_CORES)], axis=0)


kernel.last_result = None


# revision 8
# speedup vs baseline: 1.0275x; 1.0275x over previous
"""Trainium2 Bass kernel for BlockChunkedActivityRoutedNet.

Reference computation (B=4096, IN_F=4096, 8 chunks of 512, top-2 by mean|x|,
chunk-expert Linears 512->512, concat -> final Linear 1024->4096):

    xr = x.reshape(B, 8, 512)
    activities = mean(|xr|, axis=(0, 2))            # over the WHOLE batch
    i0, i1 = top2(activities)                        # descending
    h = concat(xr[:, i0] @ Wc[i0] + bc[i0], xr[:, i1] @ Wc[i1] + bc[i1])
    out = h @ W_final + b_final

Distribution: data-parallel over the batch across 8 NeuronCores (512 rows
each). Per-chunk |x| partial sums are AllReduced (tiny [1,8] collective) so
every core computes the identical top-2 routing.

Schedule (v3):
  - Head: x load as 8 x 512KB chunk DMAs on the sync HWDGE ring; per-chunk
    |.| reduces split DVE (even chunks) / ScalarE (odd) as tiles land;
    partition-reduce via ones-matmul (the FIRST op in the Tensor stream so
    nothing shallow-dep can block the AllReduce trigger chain); AllReduce
    trigger lands ~23us.
  - b_final broadcast [128,4096] and b_chunks transpose [128,32] are
    HOST-side layout prep, uploaded directly - no PE/DVE work competes with
    the trigger chain (in v2 Tile's list scheduler hoisted those matmuls
    ahead of the partition-reduce and the trigger slipped 25us).
  - W_final prefetch (8 x 1MB row-blocks) + bfin_bc load fill the
    AllReduce/barrier wait window.
  - Routing: top2 via vector.max/max_index, K=1 ones-matmul broadcast,
    iota row offsets. Selected rows gathered from a combined [4096, 1024]
    bf16 table XW (x row | W row share the row index): 8 indirect DMAs of
    [128, 1024], chunk 0 first.
  - L1 hT[s][d] via 4 k-tile matmuls into PSUM, bias added on ScalarE
    eviction to bf16. Six L2 PSUM groups (bt=0, o=0..5, kf 0-3 on hT[0])
    pre-start while chunk-1 gathers/L1 run so the PE never idles.
  - L2 loops bt outer / o inner; per bt the 8 o-groups evict (DVE add of
    b_final, cast to bf16) into one [128, 4096] staging tile, stored with a
    single 1MB DMA -> 4 output DMAs total, small tail. Host casts to fp32.
"""

import numpy as np
import ml_dtypes

import concourse.bass as bass
import concourse.bacc as bacc
import concourse.mybir as mybir
from concourse.tile import TileContext
from concourse.bass_utils import run_bass_kernel_spmd

dt = mybir.dt
P = 128

NUM_CHUNKS = 8
TOP_K = 2
IN_F = 4096
HID_F = 4096
OUT_F = 4096
B = 4096
CIN = IN_F // NUM_CHUNKS      # 512
COUT = HID_F // NUM_CHUNKS    # 512
N_CORES = 8
BS = B // N_CORES             # 512 rows per core

BT = BS // P                  # 4 batch tiles per core
KT = CIN // P                 # 4 k-tiles per selected chunk
DT_ = COUT // P               # 4 d-tiles per selected chunk
KF = TOP_K * DT_              # 8 k-tiles for the final matmul
OT = OUT_F // 512             # 8 output column tiles of 512

_cache = {}


def _build():
    nc = bacc.Bacc(num_devices=N_CORES, name="chunk_routed_net",
                   num_swdge_queues=1)

    xT = nc.dram_tensor("xT_shard", [IN_F, BS], dt.bfloat16,
                        kind="ExternalInput")
    XW = nc.dram_tensor("XW_shard", [IN_F, BS + COUT], dt.bfloat16,
                        kind="ExternalInput")
    bT_d = nc.dram_tensor("bT_host", [P, DT_ * NUM_CHUNKS], dt.float32,
                          kind="ExternalInput")
    Wf = nc.dram_tensor("W_final", [COUT * TOP_K, OUT_F], dt.bfloat16,
                        kind="ExternalInput")
    bfb = nc.dram_tensor("b_final_bc", [P, OUT_F], dt.float32,
                         kind="ExternalInput")
    out = nc.dram_tensor("out_shard", [BS, OUT_F], dt.bfloat16,
                         kind="ExternalOutput")

    with TileContext(nc) as tc:
        with tc.tile_pool(name="consts", bufs=1) as consts, \
             tc.tile_pool(name="route", bufs=1) as route, \
             tc.tile_pool(name="xl", bufs=1) as xl_pool, \
             tc.tile_pool(name="gath", bufs=1) as gath, \
             tc.tile_pool(name="hts", bufs=1) as hts, \
             tc.tile_pool(name="bfinp", bufs=1) as bfinp, \
             tc.tile_pool(name="wfs", bufs=8) as wfs, \
             tc.tile_pool(name="outs", bufs=2) as outs, \
             tc.tile_pool(name="dram", bufs=1, space="DRAM") as dram:

            # ---------------- constants ----------------
            ones_col = consts.tile([P, 1], dt.float32)     # partition reduce
            nc.vector.memset(ones_col[:], 1.0)
            ones_k1 = consts.tile([1, P], dt.float32)      # K=1 bcast matmul
            nc.vector.memset(ones_k1[:], 1.0)
            # C_W[p, j] = p + 128*j       (row offset within a chunk)
            C_W = consts.tile([P, KT], dt.int32)
            nc.gpsimd.iota(C_W[:], pattern=[[P, KT]], base=0, channel_multiplier=1)
            C_Wf = consts.tile([P, KT], dt.float32)
            nc.vector.tensor_copy(C_Wf[:], C_W[:])
            # C8[p, c] = c                (chunk-id iota along free dim)
            C8 = consts.tile([P, NUM_CHUNKS], dt.int32)
            nc.gpsimd.iota(C8[:], pattern=[[1, NUM_CHUNKS]], base=0,
                           channel_multiplier=0)
            C8f = consts.tile([P, NUM_CHUNKS], dt.float32)
            nc.vector.tensor_copy(C8f[:], C8[:])

            with tc.tile_pool(name="ps_early", bufs=2, space="PSUM") as ps_early:
                # ------------ activities from xT (chunk-aligned loads) -----
                # chunk c rows of view [1024, 2048] = xT feats 4r..4r+3; rows
                # c*128..c*128+127 are exactly chunk c -> 4KB runs/partition.
                xT_w = xT[:].rearrange("(r q) b -> r (q b)", q=4)  # [1024,2048]
                actcol = route.tile([P, NUM_CHUNKS], dt.float32)
                scr = route.tile([P, 4 * BS], dt.bfloat16)  # ACT throwaway
                xls = []
                for c in range(NUM_CHUNKS):
                    xlt = xl_pool.tile([P, 4 * BS], dt.bfloat16, tag=f"xl{c}",
                                       name=f"xl{c}")
                    nc.sync.dma_start(xlt[:], xT_w[c * P:(c + 1) * P, :])
                    xls.append(xlt)
                for c in range(NUM_CHUNKS):
                    if c % 2 == 0:
                        nc.vector.tensor_reduce(
                            actcol[:, c:c + 1], xls[c][:],
                            axis=mybir.AxisListType.X, op=mybir.AluOpType.add,
                            apply_absolute_value=True)
                    else:
                        nc.scalar.activation(
                            scr[:], xls[c][:],
                            mybir.ActivationFunctionType.Abs,
                            accum_out=actcol[:, c:c + 1])
                act_ps = ps_early.tile([1, NUM_CHUNKS], dt.float32, tag="psa")
                nc.tensor.matmul(act_ps[:], ones_col[:], actcol[:],
                                 start=True, stop=True)
                act_l = route.tile([1, NUM_CHUNKS], dt.float32)
                nc.scalar.copy(act_l[:], act_ps[:])

                # ------------ AllGather of per-core partials ------------
                # (AllGather has a lower ncfw floor than AllReduce; the 8x8
                # partition-sum happens on-chip with one tiny matmul)
                cc_in = dram.tile([1, NUM_CHUNKS], dt.float32)
                cc_out = dram.tile([N_CORES, NUM_CHUNKS], dt.float32)
                nc.sync.dma_start(cc_in[:], act_l[:])
                nc.gpsimd.collective_compute(
                    "AllGather", mybir.AluOpType.bypass,
                    replica_groups=[list(range(N_CORES))],
                    ins=[cc_in.opt()], outs=[cc_out.opt()])

                # ---- work that fills the AllReduce wait ----
                # W_final prefetch: 8 x 1MB row-blocks [128, 4096]
                wf_tiles = []
                for kf in range(KF):
                    w = wfs.tile([P, OUT_F], dt.bfloat16, tag="wf",
                                 name=f"wf{kf}")
                    nc.sync.dma_start(w[:], Wf[kf * P:(kf + 1) * P, :])
                    wf_tiles.append(w)
                # host-prepped b_final broadcast + b_chunks transpose
                bfin_bc = bfinp.tile([P, OUT_F], dt.float32)
                nc.scalar.dma_start(bfin_bc[:], bfb[:])
                bT = route.tile([P, DT_ * NUM_CHUNKS], dt.float32)
                nc.scalar.dma_start(bT[:], bT_d[:])

                ag_sb = route.tile([N_CORES, NUM_CHUNKS], dt.float32)
                nc.scalar.dma_start(ag_sb[:], cc_out[:])
                ag_ps = ps_early.tile([1, NUM_CHUNKS], dt.float32, tag="psg")
                nc.tensor.matmul(ag_ps[:], ones_col[0:N_CORES, 0:1], ag_sb[:],
                                 start=True, stop=True)
                act_g = route.tile([1, NUM_CHUNKS], dt.float32)
                nc.vector.tensor_copy(act_g[:], ag_ps[:])

                # ------------ top-2 ------------
                maxv = route.tile([1, NUM_CHUNKS], dt.float32)
                maxi = route.tile([1, NUM_CHUNKS], dt.uint32)
                nc.vector.max(maxv[:], act_g[:])
                nc.vector.max_index(maxi[:], maxv[:], act_g[:])
                maxi_f = route.tile([1, NUM_CHUNKS], dt.float32)
                nc.vector.tensor_copy(maxi_f[:], maxi[:])

                # bcast[p, j] = idx[j] on every partition (K=1 matmul)
                bc_ps = ps_early.tile([P, NUM_CHUNKS], dt.float32, tag="psc")
                nc.tensor.matmul(bc_ps[:], ones_k1[:], maxi_f[:],
                                 start=True, stop=True)
                bcast = route.tile([P, NUM_CHUNKS], dt.float32)
                nc.vector.tensor_copy(bcast[:], bc_ps[:])

            # gather offsets: offW[p, s*4+kt] = sel_s*512 + kt*128 + p
            bc512 = route.tile([P, TOP_K], dt.float32)
            nc.vector.tensor_scalar_mul(bc512[:], bcast[:, 0:TOP_K], 512.0)
            offW_f = route.tile([P, TOP_K * KT], dt.float32)
            for s in range(TOP_K):
                nc.vector.tensor_scalar(
                    offW_f[:, s * KT:(s + 1) * KT], C_Wf[:],
                    bc512[:, s:s + 1], scalar2=None, op0=mybir.AluOpType.add)
            offW = route.tile([P, TOP_K * KT], dt.int32)
            nc.vector.tensor_copy(offW[:], offW_f[:])

            # ------------ gathers from combined XW table, chunk 0 first ---
            RW = BS + COUT
            XWg = [[gath.tile([P, RW], dt.bfloat16, tag=f"xw{s}_{kt}",
                              name=f"xw{s}_{kt}")
                    for kt in range(KT)] for s in range(TOP_K)]
            for s in range(TOP_K):
                for kt in range(KT):
                    nc.gpsimd.indirect_dma_start(
                        out=XWg[s][kt][:], out_offset=None,
                        in_=XW[:],
                        in_offset=bass.IndirectOffsetOnAxis(
                            ap=offW[:, s * KT + kt:s * KT + kt + 1], axis=0))

            def xg(s, kt):
                return XWg[s][kt][:, 0:BS]          # [128, 512] x rows

            def wk(s, kt, d):
                return XWg[s][kt][:, BS + d * P:BS + (d + 1) * P]

            # chunk-bias select: bias[s][d][p] = bT[p, d*8 + sel_s]
            onehot = route.tile([P, TOP_K * NUM_CHUNKS], dt.float32)
            for s in range(TOP_K):
                nc.vector.tensor_scalar(
                    onehot[:, s * NUM_CHUNKS:(s + 1) * NUM_CHUNKS], C8f[:],
                    bcast[:, s:s + 1], scalar2=None,
                    op0=mybir.AluOpType.is_equal)
            bsel = [[route.tile([P, 1], dt.float32, tag=f"bs{s}_{d}",
                                name=f"bs{s}_{d}")
                     for d in range(DT_)] for s in range(TOP_K)]
            btmp = route.tile([P, NUM_CHUNKS], dt.float32)
            for s in range(TOP_K):
                for d in range(DT_):
                    nc.vector.tensor_tensor(
                        out=btmp[:], in0=bT[:, d * NUM_CHUNKS:(d + 1) * NUM_CHUNKS],
                        in1=onehot[:, s * NUM_CHUNKS:(s + 1) * NUM_CHUNKS],
                        op=mybir.AluOpType.mult)
                    nc.vector.tensor_reduce(
                        bsel[s][d][:], btmp[:], axis=mybir.AxisListType.X,
                        op=mybir.AluOpType.add)

            with tc.tile_pool(name="ps_h", bufs=2, space="PSUM") as ps_h, \
                 tc.tile_pool(name="ps_o", bufs=6, space="PSUM") as ps_o:
                # ------------ L1: hT[s][d] = (x_sel @ Wc[sel]).T + b -------
                hT = [[hts.tile([P, BS], dt.bfloat16, tag=f"ht{s}_{d}",
                                name=f"ht{s}_{d}")
                       for d in range(DT_)] for s in range(TOP_K)]

                def l1_chunk(s):
                    for d in range(DT_):
                        ph = ps_h.tile([P, BS], dt.float32, tag="ph",
                                       name=f"ph{s}_{d}")
                        for kt in range(KT):
                            nc.tensor.matmul(
                                ph[:], wk(s, kt, d), xg(s, kt),
                                start=(kt == 0), stop=(kt == KT - 1))
                        nc.scalar.activation(
                            hT[s][d][:], ph[:],
                            mybir.ActivationFunctionType.Identity,
                            bias=bsel[s][d][:, 0:1])

                l1_chunk(0)

                # pre-start six psum groups (bt=0, o=0..5) on chunk-0 hT
                # while chunk-1 gathers are still in flight (PE executes in
                # program order, so this fills the wait)
                PRE = [(0, 0), (0, 1), (0, 2), (0, 3), (0, 4), (0, 5)]
                pre = {}
                for (bt, po_) in PRE:
                    po = ps_o.tile([P, 512], dt.float32, tag="po",
                                   name=f"po_pre{bt}_{po_}")
                    for kf in range(DT_):
                        nc.tensor.matmul(
                            po[:], hT[0][kf][:, bt * P:(bt + 1) * P],
                            wf_tiles[kf][:, po_ * 512:(po_ + 1) * 512],
                            start=(kf == 0), stop=False)
                    pre[(bt, po_)] = po

                l1_chunk(1)

                # ------------ L2: out = h @ W_final + b_final --------------
                # bt outer / o inner; evict each o-group into a [128, 4096]
                # staging row, store once per bt (4 x 1MB output DMAs).
                for bt in range(BT):
                    ot_sb = outs.tile([P, OUT_F], dt.bfloat16, tag="ot",
                                      name=f"ot{bt}")
                    for o in range(OT):
                        osl = slice(o * 512, (o + 1) * 512)
                        if (bt, o) in pre:
                            po = pre[(bt, o)]
                            kfs = range(DT_, KF)
                        else:
                            po = ps_o.tile([P, 512], dt.float32, tag="po",
                                           name=f"po{bt}_{o}")
                            kfs = range(KF)
                        for kf in kfs:
                            s, d = divmod(kf, DT_)
                            nc.tensor.matmul(
                                po[:], hT[s][d][:, bt * P:(bt + 1) * P],
                                wf_tiles[kf][:, osl],
                                start=(kf == 0), stop=(kf == KF - 1))
                        nc.vector.tensor_tensor(
                            out=ot_sb[:, osl], in0=po[:], in1=bfin_bc[:, osl],
                            op=mybir.AluOpType.add)
                    nc.sync.dma_start(out[bt * P:(bt + 1) * P, :], ot_sb[:])
    nc.compile()
    return nc


def kernel(x, W_chunks, b_chunks, W_final, b_final):
    bf16 = ml_dtypes.bfloat16
    x = np.asarray(x, dtype=np.float32).astype(bf16)
    W_chunks = np.asarray(W_chunks, dtype=np.float32).astype(bf16)
    W_final = np.ascontiguousarray(
        np.asarray(W_final, dtype=np.float32).astype(bf16))
    b_chunks = np.asarray(b_chunks, dtype=np.float32)
    b_final = np.asarray(b_final, dtype=np.float32).reshape(OUT_F)

    # host-side layout prep (input-independent):
    # bT[p, d*8 + c] = b_chunks[c, d*128 + p]
    bT_host = np.ascontiguousarray(
        b_chunks.T.reshape(DT_, P, NUM_CHUNKS).transpose(1, 0, 2)
        .reshape(P, DT_ * NUM_CHUNKS))
    b_final_bc = np.ascontiguousarray(
        np.broadcast_to(b_final[None, :], (P, OUT_F)))

    if "nc" not in _cache:
        _cache["nc"] = _build()
    nc = _cache["nc"]

    Wc_rows = W_chunks.reshape(IN_F, COUT)          # [4096, 512] bf16
    in_maps = []
    for c in range(N_CORES):
        xT = np.ascontiguousarray(x[c * BS:(c + 1) * BS].T)  # [4096, 512]
        XW = np.concatenate([xT, Wc_rows], axis=1)           # [4096, 1024]
        in_maps.append({
            "xT_shard": xT,
            "XW_shard": np.ascontiguousarray(XW),
            "bT_host": bT_host,
            "W_final": W_final,
            "b_final_bc": b_final_bc,
        })

    res = run_bass_kernel_spmd(nc, in_maps, core_ids=list(range(N_CORES)))
    kernel.last_result = res
    return np.concatenate(
        [res.results[c]["out_shard"].astype(np.float32)
         for c in range(N_CORES)], axis=0)


kernel.last_result = None
